# revision 20
# baseline (speedup 1.0000x reference)
"""Trainium2 Bass kernel for a 4-layer IndRNN (B=32, T=2048, I=256, H=512).

Math per layer: xp = x @ W.T (+b), then h_t = relu(xp_t + w (*) h_{t-1}),
with per-channel recurrent weight w = whs[l] in [0, 1).

The nonlinear scan decomposes into two linear-style DVE scans (see the
baseline derivation): dloc = linear scan of xp with factor w, q = min-scan,
h = relu(dloc - q). DVE scans cost ~2.1 ns/element regardless of dtype, so
this kernel additionally DECIMATES TIME BY 2: both scans run at length T/2
over pair-combined inputs, and the other parity is recovered on PE/ACT.

Per channel (K = T/2), validated exactly in fp64 (sim_check.py):
    xp_e[k] = xp[2k], xp_o[k] = xp[2k+1]
    y[k]        = w*xp_e[k] + xp_o[k]        (PE proj + ACT scale + PE I-accum)
    dloc[k]     = w^2*dloc[k-1] + y[k]       (DVE scan 1, length T/2)
    u'[k]       = relu(xp_o[k]) - dloc[k]    (ACT relu + DVE subtract)
    M'[k+1]     = max(w^2*M'[k], u'[k])      (DVE scan 2;  M' = -M)
    h_odd[k]    = dloc[k] + M'[k+1]          (PE identity-matmul accumulate)
    dloc_e[k]   = w*dloc[k-1] + xp_e[k]      (PE diag-matmul accumulate)
    h_even[k]   = relu(dloc_e[k] + w*M'[k])
Since h_odd >= 0 (it is a post-relu state), relu is a no-op on it, so ONE
ACT relu over the combined PSUM region [h_odd_pre | h_even_pre] materializes
both parities in a single pass.

Only the two scans and the u' subtract run on the DVE (~5.4us per
128-channel tile); everything else lives on PE/ACT, software-pipelined one
unit ahead so PE never queues behind the scans.

Sharding: data-parallel over batch, 4 batches per core, weights replicated.
Layout on device: [H(partitions), T/2(free)] per parity per batch; the host
pre-splits time parities and pre-transposes, and re-interleaves on the way
out, so the device never pays for transposes or strided DMA.
"""

import numpy as np
from contextlib import ExitStack

import concourse.bass as bass
import concourse.tile as tile
from concourse import mybir
from concourse.bass_utils import run_bass_kernel_spmd

dt = mybir.dt
Alu = mybir.AluOpType
Act = mybir.ActivationFunctionType

B, T, I, H, L = 32, 2048, 256, 512, 4
NCORES = 8
BLOC = B // NCORES
P = 128
TH = T // 2          # decimated scan length
FC = 512             # matmul free-dim chunk (= one PSUM bank of fp32)


def build(bloc=BLOC, t=T, include_bias=False, trace_sim=False):
    """Build the per-core Bass program (SPMD; identical on all cores)."""
    assert not include_bias, "bias path not implemented (bs==0 in this problem)"
    th = t // 2
    nf = th // FC
    ki, kh, m4 = I // P, H // P, H // P

    nc = bass.Bass("TRN2", target_bir_lowering=False, debug=False,
                   num_devices=NCORES)
    xe_d = nc.dram_tensor("xe", [bloc, I, th], dt.float16, kind="ExternalInput").ap()
    xo_d = nc.dram_tensor("xo", [bloc, I, th], dt.float16, kind="ExternalInput").ap()
    w0t_d = nc.dram_tensor("w0t", [I, H], dt.float16, kind="ExternalInput").ap()
    wst_d = nc.dram_tensor("wst", [L - 1, H, H], dt.float16, kind="ExternalInput").ap()
    idm_d = nc.dram_tensor("idm", [P, P], dt.float16, kind="ExternalInput").ap()
    dg_d = nc.dram_tensor("dg", [L, m4, P, P], dt.float16, kind="ExternalInput").ap()
    w2_d = nc.dram_tensor("w2", [L, H, 1], dt.float32, kind="ExternalInput").ap()
    wv1_d = nc.dram_tensor("wv1", [L, H, 1], dt.float32, kind="ExternalInput").ap()
    # output, parity-split: [b, H, parity(0=odd,1=even), T/2]
    out_d = nc.dram_tensor("out", [bloc, H, 2, th], dt.float16,
                           kind="ExternalOutput").ap()

    with tile.TileContext(nc, trace_sim=trace_sim) as tc, ExitStack() as ctx:
        wpool = ctx.enter_context(tc.tile_pool(name="weights", bufs=1))
        xpool = ctx.enter_context(tc.tile_pool(name="xin", bufs=ki * bloc))
        hpool = ctx.enter_context(tc.tile_pool(name="h", bufs=20))
        rpool = ctx.enter_context(tc.tile_pool(name="r", bufs=2))
        tpool = ctx.enter_context(tc.tile_pool(name="t", bufs=3))
        upool = ctx.enter_context(tc.tile_pool(name="u", bufs=2))
        dpool = ctx.enter_context(tc.tile_pool(name="dloc", bufs=2))
        mpool = ctx.enter_context(tc.tile_pool(name="mmin", bufs=2))
        spool = ctx.enter_context(tc.tile_pool(name="stage", bufs=2))
        psum = ctx.enter_context(tc.tile_pool(name="psum", bufs=2, space="PSUM"))

        # ---- persistent weights ----
        wt = []   # wt[l][k] -> [128, H] fp16 lhsT
        for l in range(L):
            kprev = ki if l == 0 else kh
            tw = []
            for k in range(kprev):
                w = wpool.tile([P, H], dt.float16, tag=f"w{l}{k}")
                if l == 0:
                    nc.gpsimd.dma_start(out=w[:], in_=w0t_d[k * P:(k + 1) * P, :])
                else:
                    nc.gpsimd.dma_start(out=w[:], in_=wst_d[l - 1, k * P:(k + 1) * P, :])
                tw.append(w)
            wt.append(tw)
        idm = wpool.tile([P, P], dt.float16, tag="idm")
        nc.gpsimd.dma_start(out=idm[:], in_=idm_d)
        dg, w2, w1 = [], [], []
        for l in range(L):
            td, tv, tv1 = [], [], []
            for m in range(m4):
                d = wpool.tile([P, P], dt.float16, tag=f"dg{l}{m}")
                v = wpool.tile([P, 1], dt.float32, tag=f"w2{l}{m}")
                v1 = wpool.tile([P, 1], dt.float32, tag=f"w1{l}{m}")
                nc.gpsimd.dma_start(out=d[:], in_=dg_d[l, m, :, :])
                nc.gpsimd.dma_start(out=v[:], in_=w2_d[l, m * P:(m + 1) * P, :])
                nc.gpsimd.dma_start(out=v1[:], in_=wv1_d[l, m * P:(m + 1) * P, :])
                td.append(d)
                tv.append(v)
                tv1.append(v1)
            dg.append(td)
            w2.append(tv)
            w1.append(tv1)

        # layer-0 inputs, all batches up front
        xe0, xo0 = [], []
        for b in range(bloc):
            te, to = [], []
            for k in range(ki):
                e = xpool.tile([P, th], dt.float16, tag="xe")
                o = xpool.tile([P, th], dt.float16, tag="xo")
                nc.gpsimd.dma_start(out=e[:], in_=xe_d[b, k * P:(k + 1) * P, :])
                nc.gpsimd.dma_start(out=o[:], in_=xo_d[b, k * P:(k + 1) * P, :])
                te.append(e)
                to.append(o)
            xe0.append(te)
            xo0.append(to)

        # Non-PE instructions carry only ONE sync-wait through walrus codegen.
        # Same-engine waits merge, so each engine first "claims" every
        # DMA-loaded operand it will read, leaving real ops a single wait.
        scratch = wpool.tile([P, L * m4], dt.float32, tag="scratch")
        scr_act = wpool.tile([P, 2 * L * m4], dt.float32, tag="scr_act")
        for l in range(L):
            for m in range(m4):
                col = slice(l * m4 + m, l * m4 + m + 1)
                col2 = slice(L * m4 + l * m4 + m, L * m4 + l * m4 + m + 1)
                nc.vector.tensor_copy(scratch[:, col], w2[l][m][:, 0:1])
                nc.scalar.activation(scr_act[:, col], w2[l][m][:, 0:1], Act.Relu)
                nc.scalar.activation(scr_act[:, col2], w1[l][m][:, 0:1], Act.Relu)
        # rotating ACT-claimer scratch (fixed column would WAW itself)
        scr_rot = wpool.tile([P, 6 * bloc * L * m4], dt.float32, tag="scr_rot")
        # PE preamble: junk ldweights per weight tile so later real matmuls
        # never carry a weight-DMA wait.
        for l in range(L):
            for k in range(len(wt[l])):
                nc.tensor.ldweights(weights=wt[l][k][:, 0:P])
            for m in range(m4):
                nc.tensor.ldweights(weights=dg[l][m][:])
        nc.tensor.ldweights(weights=idm[:])

        # ---- main loop: layer-outer, batch-inner for cross-unit pipelining;
        # the last layer runs m-outer so its 2 staging tiles double-buffer.
        # Each unit's tail (block C, h_odd matmuls, combined h relu, stores)
        # is emitted inside the NEXT unit so PE never queues behind the scans.
        h = {}        # (b, l, m) -> combined [h_odd | h_even] tile [P, 2*th]
        h_ins = {}    # (b, l) -> last ACT h instruction
        stages = {}
        state = {"cnt": 0, "ci": 0, "last_mm": None, "scan1": None,
                 "last_dve": None, "last_act": None, "tail": None}
        px_readers = {}  # px slot / 4: r slot -> last reader instruction

        def dve(ins):
            # pin DVE queue order (sync=False deps cost no sem waits)
            if state["last_dve"] is not None:
                bass._add_dep_helper(ins.ins, state["last_dve"].ins,
                                     sync=False, reason="DVE program order")
            state["last_dve"] = ins
            return ins

        def act(ins):
            if state["last_act"] is not None:
                bass._add_dep_helper(ins.ins, state["last_act"].ins,
                                     sync=False, reason="ACT program order")
            state["last_act"] = ins
            return ins

        def act_claim(dep, vec):
            ci = state["ci"]
            c = act(nc.scalar.activation(scr_rot[:, ci:ci + 1],
                                         vec[:, 0:1], Act.Relu))
            state["ci"] = ci + 1
            if dep is not None:
                bass._add_dep_helper(c.ins, dep.ins, sync=True,
                                     reason="ACT claimer")
            return c

        def unit(l, b, m):
            kprev = ki if l == 0 else kh
            if l == 0:
                rhs_e = [xt[:] for xt in xe0[b]]
                rhs_o = [xt[:] for xt in xo0[b]]
            else:
                rhs_e = [h[(b, l - 1, k)][:, th:2 * th] for k in range(kh)]
                rhs_o = [h[(b, l - 1, k)][:, 0:th] for k in range(kh)]
            ms = slice(m * P, (m + 1) * P)
            # one combined psum tile: [y -> h_odd_pre | xp_e -> h_even_pre]
            px = psum.tile([P, 2 * th], dt.float32, tag="px")
            slot = state["cnt"] % 2
            state["cnt"] += 1
            claimers = []
            old = px_readers.get(slot)
            if old is not None:
                ldw = nc.tensor.ldweights(weights=wt[l][0][:, 0:P])
                bass._add_dep_helper(ldw.ins, old.ins, sync=True,
                                     reason="PE claimer for PSUM slot WAR")
                claimers.append(ldw)
            if m == 0:
                # absorb rhs producer ticks (DMA for l0; ACT h of (b, l-1))
                if l == 0:
                    for xt in (*xe0[b], *xo0[b]):
                        claimers.append(nc.tensor.ldweights(
                            weights=xt[:, 0:P]))
                else:
                    ldw = nc.tensor.ldweights(weights=wt[l][0][:, 0:P])
                    bass._add_dep_helper(ldw.ins, h_ins[(b, l - 1)].ins,
                                         sync=True,
                                         reason="PE claimer for rhs producer")
                    claimers.append(ldw)
            # PE block A: px[0:th] = W.x_o (xp_odd), px[th:] = W.x_e
            first = True
            for f in range(nf):
                fs = slice(f * FC, (f + 1) * FC)
                for k in range(kprev):
                    mm = nc.tensor.matmul(
                        px[:, fs], lhsT=wt[l][k][:, ms], rhs=rhs_o[k][:, fs],
                        start=(k == 0), stop=(k == kprev - 1))
                    if first:
                        for cl in claimers:
                            bass._add_dep_helper(
                                mm.ins, cl.ins, sync=False,
                                reason="order claimers before MMs")
                        first = False
            for f in range(nf):
                fs = slice(f * FC, (f + 1) * FC)
                for k in range(kprev):
                    nc.tensor.matmul(
                        px[:, th + f * FC:th + (f + 1) * FC],
                        lhsT=wt[l][k][:, ms], rhs=rhs_e[k][:, fs],
                        start=(k == 0), stop=(k == kprev - 1))
            # ACT r = relu(xp_odd), t = w (*) xp_e
            r = rpool.tile([P, th], dt.float16, tag="r")
            act_claim(px_readers.get(4), w2[l][m])
            r_ins = act(nc.scalar.activation(r[:], px[:, 0:th], Act.Relu))
            t_ = tpool.tile([P, th], dt.float16, tag="t")
            t_ins = act(nc.scalar.activation(t_[:], px[:, th:2 * th], Act.Copy,
                                             scale=w1[l][m][:, 0:1]))
            # previous unit's tail PE block: overlaps this unit's ACT stage
            if state["tail"] is not None:
                state["tail"][0]()
            # PE block B: px[0:th] += I.t  -> y
            ldw = nc.tensor.ldweights(weights=idm[:])
            bass._add_dep_helper(ldw.ins, t_ins.ins, sync=True,
                                 reason="PE claimer: y-accum waits ACT reads")
            first = True
            for f in range(nf):
                fs = slice(f * FC, (f + 1) * FC)
                mm = nc.tensor.matmul(
                    px[:, fs], lhsT=idm[:], rhs=t_[:, fs],
                    start=False, stop=True, skip_group_check=True)
                if first:
                    bass._add_dep_helper(mm.ins, ldw.ins, sync=False,
                                         reason="order claimer before MMs")
                    first = False
                state["last_mm"] = mm
            # previous unit's tail rest (combined h relu / stores)
            if state["tail"] is not None:
                state["tail"][1]()
                state["tail"] = None
            # DVE scan 1: dloc; out at col 2 so fp16 reads stay 4B-aligned
            dloc = dpool.tile([P, th + 2], dt.float16, tag="dloc")
            ms0 = dve(nc.vector.memset(dloc[:, 1:2], 0.0))
            bass._add_dep_helper(ms0.ins, t_ins.ins, sync=True,
                                 reason="DVE claimer: ACT r+t ticks")
            wv = w2[l][m][:, 0:1].broadcast_to((P, th))
            scan1 = dve(nc.vector.tensor_tensor_scan(
                out=dloc[:, 2:th + 2], data0=wv, data1=px[:, 0:th],
                initial=0.0, op0=Alu.mult, op1=Alu.add))
            state["scan1"] = scan1
            # DVE u' = r - dloc (feeds the max-scan for M' = -M)
            u = upool.tile([P, th], dt.float16, tag="u")
            u_ins = dve(nc.vector.tensor_tensor(
                out=u[:], in0=r[:], in1=dloc[:, 2:th + 2], op=Alu.subtract))
            px_readers[4] = u_ins
            # DVE scan 2: M'[k+1] = max(w^2 M'[k], u'[k])
            mt = mpool.tile([P, th + 2], dt.float16, tag="mmin")
            dve(nc.vector.memset(mt[:, 1:2], 0.0))
            dve(nc.vector.tensor_tensor_scan(
                out=mt[:, 2:th + 2], data0=wv, data1=u[:],
                initial=0.0, op0=Alu.mult, op1=Alu.max))
            cbox = {}

            def tail_pe():
                # block C: px[th:] += diag(w).dloc_shift + diag(w).M'_shift
                for f in range(nf):
                    fs = slice(th + f * FC, th + (f + 1) * FC)
                    nc.tensor.matmul(
                        px[:, fs], lhsT=dg[l][m][:],
                        rhs=dloc[:, 1 + f * FC:1 + f * FC + FC],
                        start=False, stop=False, skip_group_check=True)
                    cbox["c"] = nc.tensor.matmul(
                        px[:, fs], lhsT=dg[l][m][:],
                        rhs=mt[:, 1 + f * FC:1 + f * FC + FC],
                        start=False, stop=True, skip_group_check=True)
                    state["last_mm"] = cbox["c"]
                # h_odd_pre = I.dloc + I.M' into px[0:th] (free after scan1)
                for f in range(nf):
                    fs = slice(f * FC, (f + 1) * FC)
                    nc.tensor.matmul(
                        px[:, fs], lhsT=idm[:],
                        rhs=dloc[:, 2 + f * FC:2 + f * FC + FC],
                        start=True, stop=False, skip_group_check=True)
                    cbox["h"] = nc.tensor.matmul(
                        px[:, fs], lhsT=idm[:],
                        rhs=mt[:, 2 + f * FC:2 + f * FC + FC],
                        start=False, stop=True, skip_group_check=True)
                    state["last_mm"] = cbox["h"]

            def tail_rest():
                # ACT claimer: absorb the last PE tick so the combined h
                # relu keeps only its own-engine ordering wait
                act_claim(cbox["h"], w2[l][m])
                if l < L - 1:
                    ht = hpool.tile([P, 2 * th], dt.float16, tag="h")
                    h_ins[(b, l)] = act(nc.scalar.activation(
                        ht[:], px[:], Act.Relu))
                    h[(b, l, m)] = ht
                    px_readers[slot] = h_ins[(b, l)]
                else:
                    # final layer: ACT writes [h_odd | h_even] straight into
                    # the batch-pair staging tile, one DMA per (b-pair, m)
                    if b % 2 == 0:
                        st = spool.tile([P, 2 * t], dt.float16, tag="stage")
                        stages[m] = st
                        # first toucher claims the store-DMA WAR tick
                        act(nc.scalar.activation(st[:, 0:1],
                                                 w2[l][m][:, 0:1], Act.Relu))
                    st = stages[m]
                    off = (b % 2) * t
                    hst = act(nc.scalar.activation(
                        st[:, off:off + t], px[:], Act.Relu))
                    px_readers[slot] = hst
                    if b % 2 == 1:
                        dst = out_d[b - 1:b + 1, ms, :, :]
                        nc.sync.dma_start(
                            out=dst.rearrange("b p r t -> p b r t"),
                            in_=st[:].rearrange("p (b r t) -> p b r t",
                                                b=2, r=2))
            state["tail"] = (tail_pe, tail_rest)

        for l in range(L - 1):
            for b in range(bloc):
                for m in range(m4):
                    unit(l, b, m)
        for m in range(m4):
            for b in range(bloc):
                unit(L - 1, b, m)
        state["tail"][0]()
        # final flush has no successor unit to absorb the last scan tick
        act_claim(state["scan1"], w2[0][0])
        state["tail"][1]()
        state["tail"] = None

        # ---- tail pre-drain (see baseline): absorb every DMA queue and
        # engine tick so the auto kernel-tail drain ends at zero waits.
        tail_deps = [i for i in nc.inst_map.values()
                     if type(i).__name__ == "InstDMACopy"]
        snap = list(nc.inst_map.values())
        compute_tys = {"InstTensorScalarPtr", "InstTensorTensor",
                       "InstActivation", "InstTensorCopy", "InstMemset"}
        for eng in ("DVE", "Activation"):
            last_e = [i for i in snap
                      if str(getattr(i, "engine", "")).endswith(eng)
                      and type(i).__name__ in compute_tys]
            if last_e:
                tail_deps.append(last_e[-1])
        tail_deps += [state["last_mm"].ins, state["scan1"].ins]
        for depi in tail_deps:
            dr = nc.sync.drain(fusable=False)
            bass._add_dep_helper(dr.ins, depi, sync=True,
                                 reason="tail pre-drain absorber")
    _assert_wait_budget(nc)
    return nc


_MULTI_WAIT_OK = {"InstDrain",
                  "InstEventSemaphore", "InstUnconditionalBranch",
                  "InstRegisterMove", "InstISA", "InstTensorLoad",
                  "InstTensorSave"}


def _assert_wait_budget(nc):
    bad = []
    for name, inst in nc.inst_map.items():
        ty = type(inst).__name__
        if ty in _MULTI_WAIT_OK:
            continue
        w = inst.sync_info.on_wait if inst.sync_info else []
        if len(w) > 1:
            bad.append((name, ty,
                        [f"{x.ant_name}>={x.wait_value}" for x in w]))
    if bad:
        raise RuntimeError(
            f"{len(bad)} instructions exceed the 1-sync-wait TPB limit, "
            f"first few: {bad[:5]}")


def _prep_core_inputs(Input, W0, Ws, bs, whs, core):
    """Host-side staging for one core: shard batch, transpose + parity-split
    the layer-0 input, lhsT weights, diag matrices, w and w^2 vectors."""
    bsl = slice(core * BLOC, (core + 1) * BLOC)
    xT = Input[bsl].transpose(0, 2, 1).astype(np.float16)  # [bloc, I, T]
    w0t = W0.T.astype(np.float16)                          # [I, H]
    wst = Ws.transpose(0, 2, 1).astype(np.float16)         # [L-1, H, H]
    whsf = whs.astype(np.float32)                          # [L, H]
    m4 = H // P
    dgm = np.zeros((L, m4, P, P), np.float16)
    for l in range(L):
        for m in range(m4):
            blk = whsf[l, m * P:(m + 1) * P]
            np.fill_diagonal(dgm[l, m], blk.astype(np.float16))
    return {
        "xe": np.ascontiguousarray(xT[:, :, 0::2]),
        "xo": np.ascontiguousarray(xT[:, :, 1::2]),
        "w0t": np.ascontiguousarray(w0t),
        "wst": np.ascontiguousarray(wst),
        "idm": np.eye(P, dtype=np.float16),
        "dg": dgm,
        "w2": np.ascontiguousarray((whsf * whsf)[:, :, None]),
        "wv1": np.ascontiguousarray(whsf[:, :, None]),
    }


def kernel(Input, W0, Ws, bs, whs):
    include_bias = bool(np.any(bs != 0))
    nc = build(include_bias=include_bias)
    in_maps = [_prep_core_inputs(Input, W0, Ws, bs, whs, r)
               for r in range(NCORES)]
    res = run_bass_kernel_spmd(nc, in_maps, core_ids=list(range(NCORES)))
    parts = [res.results[r]["out"] for r in range(NCORES)]  # [BLOC, H, 2, T/2]
    po = np.concatenate(parts, axis=0)  # [B, H, 2, T/2]; 0=odd, 1=even
    full = np.empty((B, H, T), np.float16)
    full[:, :, 1::2] = po[:, :, 0, :]
    full[:, :, 0::2] = po[:, :, 1, :]
    return np.ascontiguousarray(full.transpose(0, 2, 1)).astype(np.float32)


# revision 33
# speedup vs baseline: 1.8005x; 1.8005x over previous
"""Trainium2 Bass kernel for a 4-layer IndRNN (B=32, T=2048, I=256, H=512).

Math per layer: xp = x @ W.T (+b), then h_t = relu(xp_t + w (*) h_{t-1}),
with per-channel recurrent weight w = whs[l] in [0, 1).

The nonlinear scan decomposes into two linear-style DVE scans (see the
baseline derivation): dloc = linear scan of xp with factor w, q = min-scan,
h = relu(dloc - q). DVE scans cost ~2.1 ns/element regardless of dtype, so
this kernel additionally DECIMATES TIME BY 2: both scans run at length T/2
over pair-combined inputs, and the other parity is recovered on PE/ACT.

Per channel (K = T/2), validated exactly in fp64 (sim_check.py):
    xp_e[k] = xp[2k], xp_o[k] = xp[2k+1]
    y[k]        = w*xp_e[k] + xp_o[k]        (PE proj + ACT scale + PE I-accum)
    dloc[k]     = w^2*dloc[k-1] + y[k]       (DVE scan 1, length T/2)
    u'[k]       = relu(xp_o[k]) - dloc[k]    (ACT relu + DVE subtract)
    M'[k+1]     = max(w^2*M'[k], u'[k])      (DVE scan 2;  M' = -M)
    h_odd[k]    = dloc[k] + M'[k+1]          (PE identity-matmul accumulate)
    dloc_e[k]   = w*dloc[k-1] + xp_e[k]      (PE diag-matmul accumulate)
    h_even[k]   = relu(dloc_e[k] + w*M'[k])
Since h_odd >= 0 (it is a post-relu state), relu is a no-op on it, so ONE
ACT relu over the combined PSUM region [h_odd_pre | h_even_pre] materializes
both parities in a single pass.

Only the two scans and the u' subtract run on the DVE (~5.4us per
128-channel tile); everything else lives on PE/ACT, software-pipelined one
unit ahead so PE never queues behind the scans.

Sharding: data-parallel over batch, 4 batches per core, weights replicated.
Layout on device: [H(partitions), T/2(free)] per parity per batch; the host
pre-splits time parities and pre-transposes, and re-interleaves on the way
out, so the device never pays for transposes or strided DMA.
"""

import numpy as np
from contextlib import ExitStack

import concourse.bass as bass
import concourse.tile as tile
from concourse import mybir
from concourse.bass_utils import run_bass_kernel_spmd

dt = mybir.dt
Alu = mybir.AluOpType
Act = mybir.ActivationFunctionType

B, T, I, H, L = 32, 2048, 256, 512, 4
NCORES = 8
BLOC = B // NCORES
P = 128
TH = T // 2          # decimated scan length
FC = 512             # matmul free-dim chunk (= one PSUM bank of fp32)


def build(bloc=BLOC, t=T, include_bias=False, trace_sim=False):
    """Build the per-core Bass program (SPMD; identical on all cores)."""
    assert not include_bias, "bias path not implemented (bs==0 in this problem)"
    th = t // 2
    nf = th // FC
    ki, kh, m4 = I // P, H // P, H // P

    nc = bass.Bass("TRN2", target_bir_lowering=False, debug=False,
                   num_devices=NCORES)
    xe_d = nc.dram_tensor("xe", [bloc, I, th], dt.float16, kind="ExternalInput").ap()
    xo_d = nc.dram_tensor("xo", [bloc, I, th], dt.float16, kind="ExternalInput").ap()
    w0t_d = nc.dram_tensor("w0t", [I, H], dt.float16, kind="ExternalInput").ap()
    wst_d = nc.dram_tensor("wst", [L - 1, H, H], dt.float16, kind="ExternalInput").ap()
    idm_d = nc.dram_tensor("idm", [P, P], dt.float16, kind="ExternalInput").ap()
    dg_d = nc.dram_tensor("dg", [P, L * m4 * P], dt.float16, kind="ExternalInput").ap()
    w2_d = nc.dram_tensor("w2", [P, L * m4], dt.float32, kind="ExternalInput").ap()
    wv1_d = nc.dram_tensor("wv1", [P, L * m4], dt.float32, kind="ExternalInput").ap()
    # output, parity-split: [b, H, parity(0=odd,1=even), T/2]
    out_d = nc.dram_tensor("out", [bloc, H, 2, th], dt.float16,
                           kind="ExternalOutput").ap()

    with tile.TileContext(nc, trace_sim=trace_sim) as tc, ExitStack() as ctx:
        wpool = ctx.enter_context(tc.tile_pool(name="weights", bufs=1))
        xpool = ctx.enter_context(tc.tile_pool(name="xin", bufs=ki * bloc))
        hpool = ctx.enter_context(tc.tile_pool(name="h", bufs=20))
        rpool = ctx.enter_context(tc.tile_pool(name="r", bufs=2))
        tpool = ctx.enter_context(tc.tile_pool(name="t", bufs=3))
        upool = ctx.enter_context(tc.tile_pool(name="u", bufs=2))
        dpool = ctx.enter_context(tc.tile_pool(name="dloc", bufs=2))
        mpool = ctx.enter_context(tc.tile_pool(name="mmin", bufs=2))
        spool = ctx.enter_context(tc.tile_pool(name="stage", bufs=2))
        psum = ctx.enter_context(tc.tile_pool(name="psum", bufs=2, space="PSUM"))

        # ---- persistent weights (small scan/diag operands first, batched
        # into single DMAs, so the DVE/ACT preamble claimers run early) ----
        idm = wpool.tile([P, P], dt.float16, tag="idm")
        nc.gpsimd.dma_start(out=idm[:], in_=idm_d)
        w2t = wpool.tile([P, L * m4], dt.float32, tag="w2t")
        w1t = wpool.tile([P, L * m4], dt.float32, tag="w1t")
        dgt = wpool.tile([P, L * m4 * P], dt.float16, tag="dgt")
        nc.gpsimd.dma_start(out=w2t[:], in_=w2_d)
        nc.gpsimd.dma_start(out=w1t[:], in_=wv1_d)
        nc.gpsimd.dma_start(out=dgt[:], in_=dg_d)
        dg, w2, w1 = [], [], []
        for l in range(L):
            td, tv, tv1 = [], [], []
            for m in range(m4):
                i = l * m4 + m
                td.append(dgt[:, i * P:(i + 1) * P])
                tv.append(w2t[:, i:i + 1])
                tv1.append(w1t[:, i:i + 1])
            dg.append(td)
            w2.append(tv)
            w1.append(tv1)
        wt = []   # wt[l][k] -> [128, H] fp16 lhsT
        for l in range(L):
            kprev = ki if l == 0 else kh
            tw = []
            for k in range(kprev):
                w = wpool.tile([P, H], dt.float16, tag=f"w{l}{k}")
                if l == 0:
                    nc.gpsimd.dma_start(out=w[:], in_=w0t_d[k * P:(k + 1) * P, :])
                else:
                    nc.gpsimd.dma_start(out=w[:], in_=wst_d[l - 1, k * P:(k + 1) * P, :])
                tw.append(w)
            wt.append(tw)
        # layer-0 inputs, all batches up front
        xe0, xo0 = [], []
        first_x_dma = None
        for b in range(bloc):
            te, to = [], []
            for k in range(ki):
                e = xpool.tile([P, th], dt.float16, tag="xe")
                o = xpool.tile([P, th], dt.float16, tag="xo")
                dd = nc.gpsimd.dma_start(out=e[:], in_=xe_d[b, k * P:(k + 1) * P, :])
                if first_x_dma is None:
                    first_x_dma = dd
                nc.gpsimd.dma_start(out=o[:], in_=xo_d[b, k * P:(k + 1) * P, :])
                te.append(e)
                to.append(o)
            xe0.append(te)
            xo0.append(to)

        # Non-PE instructions carry only ONE sync-wait through walrus codegen.
        # Same-engine waits merge, so each engine first "claims" every
        # DMA-loaded operand it will read, leaving real ops a single wait.
        scratch = wpool.tile([P, L * m4], dt.float32, tag="scratch")
        scr_act = wpool.tile([P, 2 * L * m4], dt.float32, tag="scr_act")
        for l in range(L):
            for m in range(m4):
                col = slice(l * m4 + m, l * m4 + m + 1)
                col2 = slice(L * m4 + l * m4 + m, L * m4 + l * m4 + m + 1)
                nc.vector.tensor_copy(scratch[:, col], w2[l][m])
                nc.scalar.activation(scr_act[:, col], w2[l][m], Act.Relu)
                nc.scalar.activation(scr_act[:, col2], w1[l][m], Act.Relu)
        # rotating ACT-claimer scratch (fixed column would WAW itself)
        scr_rot = wpool.tile([P, 6 * bloc * L * m4], dt.float32, tag="scr_rot")
        # PE preamble: junk ldweights per weight tile so later real matmuls
        # never carry a weight-DMA wait.
        for l in range(L):
            for k in range(len(wt[l])):
                nc.tensor.ldweights(weights=wt[l][k][:, 0:P])
            for m in range(m4):
                nc.tensor.ldweights(weights=dg[l][m])
        nc.tensor.ldweights(weights=idm[:])

        # PE warmup: ~4us of junk matmuls so the tensor array reaches its
        # full p-state clock before the first real projection
        wrm = psum.tile([P, TH], dt.float32, tag="py")
        for i in range(18):
            wmm = nc.tensor.matmul(wrm[:, 0:FC], lhsT=wt[0][0][:, 0:P],
                                   rhs=wt[0][0][:, 0:FC], start=True,
                                   stop=True)
            if i == 0:
                bass._add_dep_helper(wmm.ins, first_x_dma.ins, sync=True,
                                     reason="warmup fires as x loads land")
        # ---- main loop: layer-outer, batch-inner for cross-unit pipelining;
        # the last layer runs m-outer so its 2 staging tiles double-buffer.
        # Each unit's tail (block C, h_odd matmuls, combined h relu, stores)
        # is emitted inside the NEXT unit so PE never queues behind the scans.
        h = {}        # (b, l, m) -> combined [h_odd | h_even] tile [P, 2*th]
        ho_ins = {}   # (b, l) -> last DVE h_odd instruction
        he_ins = {}   # (b, l) -> last ACT h_even instruction
        stages = {}
        state = {"cnt": 0, "ci": 0, "last_mm": None, "scan1": None,
                 "last_dve": None, "last_act": None, "last_pe": None,
                 "tail": None}
        px_readers = {}  # px slot / 4: r slot -> last reader instruction

        def dve(ins):
            # pin DVE queue order (sync=False deps cost no sem waits)
            if state["last_dve"] is not None:
                bass._add_dep_helper(ins.ins, state["last_dve"].ins,
                                     sync=False, reason="DVE program order")
            state["last_dve"] = ins
            return ins

        def act(ins):
            if state["last_act"] is not None:
                bass._add_dep_helper(ins.ins, state["last_act"].ins,
                                     sync=False, reason="ACT program order")
            state["last_act"] = ins
            return ins

        def pe(ins):
            if state["last_pe"] is not None:
                bass._add_dep_helper(ins.ins, state["last_pe"].ins,
                                     sync=False, reason="PE program order")
            state["last_pe"] = ins
            return ins

        def act_claim(dep, vec):
            ci = state["ci"]
            c = act(nc.scalar.activation(scr_rot[:, ci:ci + 1],
                                         vec, Act.Relu))
            state["ci"] = ci + 1
            if dep is not None:
                bass._add_dep_helper(c.ins, dep.ins, sync=True,
                                     reason="ACT claimer")
            return c

        def unit(l, b, m):
            kprev = ki if l == 0 else kh
            if l == 0:
                rhs_e = [xt[:] for xt in xe0[b]]
                rhs_o = [xt[:] for xt in xo0[b]]
            else:
                rhs_e = [h[(b, l - 1, k)][:, th:2 * th] for k in range(kh)]
                rhs_o = [h[(b, l - 1, k)][:, 0:th] for k in range(kh)]
            ms = slice(m * P, (m + 1) * P)
            # two psum tiles; py frees right after scan1 so the next unit's
            # odd-projection can start while this unit is still scanning
            py = psum.tile([P, th], dt.float32, tag="py")
            pe_ = psum.tile([P, th], dt.float32, tag="pe")
            slot = state["cnt"] % 2
            state["cnt"] += 1
            claimers = []
            for sl, rd in ((slot, px_readers.get(slot)),
                           (2 + slot, px_readers.get(2 + slot))):
                if rd is not None:
                    ldw = pe(nc.tensor.ldweights(weights=wt[l][0][:, 0:P]))
                    bass._add_dep_helper(ldw.ins, rd.ins, sync=True,
                                         reason="PE claimer for PSUM slot WAR")
                    claimers.append(ldw)
            if m == 0:
                # absorb rhs producer ticks (DMA for l0; DVE h_odd and ACT
                # h_even of (b, l-1) otherwise)
                if l == 0:
                    for xt in (*xe0[b], *xo0[b]):
                        claimers.append(pe(nc.tensor.ldweights(
                            weights=xt[:, 0:P])))
                else:
                    for dep in (ho_ins[(b, l - 1)], he_ins[(b, l - 1)]):
                        ldw = pe(nc.tensor.ldweights(weights=wt[l][0][:, 0:P]))
                        bass._add_dep_helper(
                            ldw.ins, dep.ins, sync=True,
                            reason="PE claimer for rhs producers")
                        claimers.append(ldw)
            # PE block A: py = W.x_o (xp_odd), pe = W.x_e
            first = True
            last_g1 = None
            for f in range(nf):
                fs = slice(f * FC, (f + 1) * FC)
                for k in range(kprev):
                    mm = pe(nc.tensor.matmul(
                        py[:, fs], lhsT=wt[l][k][:, ms], rhs=rhs_o[k][:, fs],
                        start=(k == 0), stop=(k == kprev - 1)))
                    if first:
                        for cl in claimers:
                            bass._add_dep_helper(
                                mm.ins, cl.ins, sync=False,
                                reason="order claimers before MMs")
                        first = False
                    last_g1 = mm
            for f in range(nf):
                fs = slice(f * FC, (f + 1) * FC)
                for k in range(kprev):
                    pe(nc.tensor.matmul(
                        pe_[:, fs], lhsT=wt[l][k][:, ms], rhs=rhs_e[k][:, fs],
                        start=(k == 0), stop=(k == kprev - 1)))
            # ACT r = relu(xp_odd), t = w (*) xp_e
            r = rpool.tile([P, th], dt.float16, tag="r")
            act_claim(px_readers.get(4 + slot), w2[l][m])
            if l == L - 1:
                # no h_even ACT op window on the last layer; absorb the PE
                # group-1 tick so r keeps a single wait
                act_claim(last_g1, w1[l][m])
            r_ins = act(nc.scalar.activation(r[:], py[:], Act.Relu))
            t_ = tpool.tile([P, th], dt.float16, tag="t")
            t_ins = act(nc.scalar.activation(t_[:], pe_[:], Act.Copy,
                                             scale=w1[l][m]))
            # previous unit's tail PE block: overlaps this unit's ACT stage
            if state["tail"] is not None:
                state["tail"][0]()
            # PE block B: py += I.t  -> y
            ldw = pe(nc.tensor.ldweights(weights=idm[:]))
            bass._add_dep_helper(ldw.ins, t_ins.ins, sync=True,
                                 reason="PE claimer: y-accum waits ACT reads")
            first = True
            for f in range(nf):
                fs = slice(f * FC, (f + 1) * FC)
                mm = pe(nc.tensor.matmul(
                    py[:, fs], lhsT=idm[:], rhs=t_[:, fs],
                    start=False, stop=True, skip_group_check=True))
                if first:
                    bass._add_dep_helper(mm.ins, ldw.ins, sync=False,
                                         reason="order claimer before MMs")
                    first = False
                state["last_mm"] = mm
            # previous unit's tail rest (h writes / stores)
            if state["tail"] is not None:
                state["tail"][1]()
                state["tail"] = None
            # DVE scan 1: dloc; out at col 2 so fp16 reads stay 4B-aligned
            dloc = dpool.tile([P, th + 2], dt.float16, tag="dloc")
            ms0 = dve(nc.vector.memset(dloc[:, 1:2], 0.0))
            bass._add_dep_helper(ms0.ins, r_ins.ins, sync=True,
                                 reason="DVE claimer: ACT r tick")
            wv = w2[l][m].broadcast_to((P, th))
            scan1 = dve(nc.vector.tensor_tensor_scan(
                out=dloc[:, 2:th + 2], data0=wv, data1=py[:],
                initial=0.0, op0=Alu.mult, op1=Alu.add))
            state["scan1"] = scan1
            px_readers[slot] = scan1
            # DVE u' = r - dloc (feeds the max-scan for M' = -M)
            u = upool.tile([P, th], dt.float16, tag="u")
            u_ins = dve(nc.vector.tensor_tensor(
                out=u[:], in0=r[:], in1=dloc[:, 2:th + 2], op=Alu.subtract))
            px_readers[4 + slot] = u_ins
            # DVE scan 2: M'[k+1] = max(w^2 M'[k], u'[k])
            mt = mpool.tile([P, th + 2], dt.float16, tag="mmin")
            dve(nc.vector.memset(mt[:, 1:2], 0.0))
            dve(nc.vector.tensor_tensor_scan(
                out=mt[:, 2:th + 2], data0=wv, data1=u[:],
                initial=0.0, op0=Alu.mult, op1=Alu.max))
            cbox = {}

            def tail_pe():
                # block C: pe += diag(w).dloc_shift + diag(w).M'_shift
                for f in range(nf):
                    fs = slice(f * FC, (f + 1) * FC)
                    pe(nc.tensor.matmul(
                        pe_[:, fs], lhsT=dg[l][m],
                        rhs=dloc[:, 1 + f * FC:1 + f * FC + FC],
                        start=False, stop=False, skip_group_check=True))
                    cbox["c"] = pe(nc.tensor.matmul(
                        pe_[:, fs], lhsT=dg[l][m],
                        rhs=mt[:, 1 + f * FC:1 + f * FC + FC],
                        start=False, stop=True, skip_group_check=True))
                    state["last_mm"] = cbox["c"]

            def tail_rest():
                if l < L - 1:
                    ht = hpool.tile([P, 2 * th], dt.float16, tag="h")
                    # h_odd = dloc + M' on DVE into ht[0:th]
                    ho_i = dve(nc.vector.tensor_tensor(
                        out=ht[:, 0:th], in0=dloc[:, 2:th + 2],
                        in1=mt[:, 2:th + 2], op=Alu.add))
                    # ACT claimer: absorb the PE block-C tick so h_even
                    # keeps only its own-engine ordering wait
                    act_claim(cbox["c"], w2[l][m])
                    he_i = act(nc.scalar.activation(ht[:, th:2 * th], pe_[:],
                                                    Act.Relu))
                    px_readers[2 + slot] = he_i
                    ho_ins[(b, l)] = ho_i
                    he_ins[(b, l)] = he_i
                    h[(b, l, m)] = ht
                else:
                    # final layer: both halves written by DVE into the
                    # batch-pair staging tile; h_even read directly from
                    # PSUM so the store carries a single DVE wait
                    if b % 2 == 0:
                        st = spool.tile([P, 2 * t], dt.float16, tag="stage")
                        stages[m] = st
                        # first toucher claims the store-DMA WAR tick
                        dve(nc.vector.memset(st[:, 0:1], 0.0))
                    st = stages[m]
                    off = (b % 2) * t
                    cs = state["ci"]
                    cdve = dve(nc.vector.memset(
                        scratch[:, (cs % (L * m4)):(cs % (L * m4)) + 1], 0.0))
                    bass._add_dep_helper(cdve.ins, cbox["c"].ins, sync=True,
                                         reason="DVE claimer: PE blockC tick")
                    ev = dve(nc.vector.tensor_scalar_max(
                        st[:, off + th:off + t], pe_[:], 0.0))
                    px_readers[2 + slot] = ev
                    odd = dve(nc.vector.tensor_tensor(
                        out=st[:, off:off + th], in0=dloc[:, 2:th + 2],
                        in1=mt[:, 2:th + 2], op=Alu.add))
                    if b % 2 == 1:
                        dst = out_d[b - 1:b + 1, ms, :, :]
                        nc.sync.dma_start(
                            out=dst.rearrange("b p r t -> p b r t"),
                            in_=st[:].rearrange("p (b r t) -> p b r t",
                                                b=2, r=2))
            state["tail"] = (tail_pe, tail_rest)

        for l in range(L - 1):
            for b in range(bloc):
                for m in range(m4):
                    unit(l, b, m)
        for m in range(m4):
            for b in range(bloc):
                unit(L - 1, b, m)
        state["tail"][0]()
        # final flush has no successor unit to absorb the last scan tick
        act_claim(state["scan1"], w2[0][0])
        state["tail"][1]()
        state["tail"] = None

        # ---- tail pre-drain (see baseline): absorb every DMA queue and
        # engine tick so the auto kernel-tail drain ends at zero waits.
        tail_deps = [i for i in nc.inst_map.values()
                     if type(i).__name__ == "InstDMACopy"]
        snap = list(nc.inst_map.values())
        compute_tys = {"InstTensorScalarPtr", "InstTensorTensor",
                       "InstActivation", "InstTensorCopy", "InstMemset"}
        for eng in ("DVE", "Activation"):
            last_e = [i for i in snap
                      if str(getattr(i, "engine", "")).endswith(eng)
                      and type(i).__name__ in compute_tys]
            if last_e:
                tail_deps.append(last_e[-1])
        tail_deps += [state["last_mm"].ins, state["scan1"].ins]
        for depi in tail_deps:
            dr = nc.sync.drain(fusable=False)
            bass._add_dep_helper(dr.ins, depi, sync=True,
                                 reason="tail pre-drain absorber")
    _assert_wait_budget(nc)
    return nc


_MULTI_WAIT_OK = {"InstDrain",
                  "InstEventSemaphore", "InstUnconditionalBranch",
                  "InstRegisterMove", "InstISA", "InstTensorLoad",
                  "InstTensorSave"}


def _assert_wait_budget(nc):
    bad = []
    for name, inst in nc.inst_map.items():
        ty = type(inst).__name__
        if ty in _MULTI_WAIT_OK:
            continue
        w = inst.sync_info.on_wait if inst.sync_info else []
        if len(w) > 1:
            bad.append((name, ty,
                        [f"{x.ant_name}>={x.wait_value}" for x in w]))
    if bad:
        raise RuntimeError(
            f"{len(bad)} instructions exceed the 1-sync-wait TPB limit, "
            f"first few: {bad[:5]}")


def _prep_core_inputs(Input, W0, Ws, bs, whs, core):
    """Host-side staging for one core: shard batch, transpose + parity-split
    the layer-0 input, lhsT weights, diag matrices, w and w^2 vectors."""
    bsl = slice(core * BLOC, (core + 1) * BLOC)
    xT = Input[bsl].transpose(0, 2, 1).astype(np.float16)  # [bloc, I, T]
    w0t = W0.T.astype(np.float16)                          # [I, H]
    wst = Ws.transpose(0, 2, 1).astype(np.float16)         # [L-1, H, H]
    whsf = whs.astype(np.float32)                          # [L, H]
    m4 = H // P
    dgm = np.zeros((L, m4, P, P), np.float16)
    for l in range(L):
        for m in range(m4):
            blk = whsf[l, m * P:(m + 1) * P]
            np.fill_diagonal(dgm[l, m], blk.astype(np.float16))
    # partition-major small operands: [p, l*m4(*P)] so each loads in one DMA
    dg_pm = np.ascontiguousarray(
        dgm.transpose(2, 0, 1, 3).reshape(P, L * m4 * P))
    w2_pm = np.ascontiguousarray(
        (whsf * whsf).reshape(L, m4, P).transpose(2, 0, 1).reshape(P, L * m4))
    w1_pm = np.ascontiguousarray(
        whsf.reshape(L, m4, P).transpose(2, 0, 1).reshape(P, L * m4))
    return {
        "xe": np.ascontiguousarray(xT[:, :, 0::2]),
        "xo": np.ascontiguousarray(xT[:, :, 1::2]),
        "w0t": np.ascontiguousarray(w0t),
        "wst": np.ascontiguousarray(wst),
        "idm": np.eye(P, dtype=np.float16),
        "dg": dg_pm,
        "w2": w2_pm,
        "wv1": w1_pm,
    }


def kernel(Input, W0, Ws, bs, whs):
    include_bias = bool(np.any(bs != 0))
    nc = build(include_bias=include_bias)
    in_maps = [_prep_core_inputs(Input, W0, Ws, bs, whs, r)
               for r in range(NCORES)]
    res = run_bass_kernel_spmd(nc, in_maps, core_ids=list(range(NCORES)))
    parts = [res.results[r]["out"] for r in range(NCORES)]  # [BLOC, H, 2, T/2]
    po = np.concatenate(parts, axis=0)  # [B, H, 2, T/2]; 0=odd, 1=even
    full = np.empty((B, H, T), np.float16)
    full[:, :, 1::2] = po[:, :, 0, :]
    full[:, :, 0::2] = po[:, :, 1, :]
    return np.ascontiguousarray(full.transpose(0, 2, 1)).astype(np.float32)


# revision 35
# speedup vs baseline: 1.8096x; 1.0050x over previous
"""Trainium2 Bass kernel for a 4-layer IndRNN (B=32, T=2048, I=256, H=512).

Math per layer: xp = x @ W.T (+b), then h_t = relu(xp_t + w (*) h_{t-1}),
with per-channel recurrent weight w = whs[l] in [0, 1).

The nonlinear scan decomposes into two linear-style DVE scans (see the
baseline derivation): dloc = linear scan of xp with factor w, q = min-scan,
h = relu(dloc - q). DVE scans cost ~2.1 ns/element regardless of dtype, so
this kernel additionally DECIMATES TIME BY 2: both scans run at length T/2
over pair-combined inputs, and the other parity is recovered on PE/ACT.

Per channel (K = T/2), validated exactly in fp64 (sim_check.py):
    xp_e[k] = xp[2k], xp_o[k] = xp[2k+1]
    y[k]        = w*xp_e[k] + xp_o[k]        (PE proj + ACT scale + PE I-accum)
    dloc[k]     = w^2*dloc[k-1] + y[k]       (DVE scan 1, length T/2)
    u'[k]       = relu(xp_o[k]) - dloc[k]    (ACT relu + DVE subtract)
    M'[k+1]     = max(w^2*M'[k], u'[k])      (DVE scan 2;  M' = -M)
    h_odd[k]    = dloc[k] + M'[k+1]          (PE identity-matmul accumulate)
    dloc_e[k]   = w*dloc[k-1] + xp_e[k]      (PE diag-matmul accumulate)
    h_even[k]   = relu(dloc_e[k] + w*M'[k])
Since h_odd >= 0 (it is a post-relu state), relu is a no-op on it, so ONE
ACT relu over the combined PSUM region [h_odd_pre | h_even_pre] materializes
both parities in a single pass.

Only the two scans and the u' subtract run on the DVE (~5.4us per
128-channel tile); everything else lives on PE/ACT, software-pipelined one
unit ahead so PE never queues behind the scans.

Sharding: data-parallel over batch, 4 batches per core, weights replicated.
Layout on device: [H(partitions), T/2(free)] per parity per batch; the host
pre-splits time parities and pre-transposes, and re-interleaves on the way
out, so the device never pays for transposes or strided DMA.
"""

import numpy as np
from contextlib import ExitStack

import concourse.bass as bass
import concourse.tile as tile
from concourse import mybir
from concourse.bass_utils import run_bass_kernel_spmd

dt = mybir.dt
Alu = mybir.AluOpType
Act = mybir.ActivationFunctionType

B, T, I, H, L = 32, 2048, 256, 512, 4
NCORES = 8
BLOC = B // NCORES
P = 128
TH = T // 2          # decimated scan length
FC = 512             # matmul free-dim chunk (= one PSUM bank of fp32)


def build(bloc=BLOC, t=T, include_bias=False, trace_sim=False):
    """Build the per-core Bass program (SPMD; identical on all cores)."""
    assert not include_bias, "bias path not implemented (bs==0 in this problem)"
    th = t // 2
    nf = th // FC
    ki, kh, m4 = I // P, H // P, H // P

    nc = bass.Bass("TRN2", target_bir_lowering=False, debug=False,
                   num_devices=NCORES)
    xe_d = nc.dram_tensor("xe", [bloc, I, th], dt.float16, kind="ExternalInput").ap()
    xo_d = nc.dram_tensor("xo", [bloc, I, th], dt.float16, kind="ExternalInput").ap()
    w0t_d = nc.dram_tensor("w0t", [I, H], dt.float16, kind="ExternalInput").ap()
    wst_d = nc.dram_tensor("wst", [L - 1, H, H], dt.float16, kind="ExternalInput").ap()
    idm_d = nc.dram_tensor("idm", [P, P], dt.float16, kind="ExternalInput").ap()
    dg_d = nc.dram_tensor("dg", [P, L * m4 * P], dt.float16, kind="ExternalInput").ap()
    w2_d = nc.dram_tensor("w2", [P, L * m4], dt.float32, kind="ExternalInput").ap()
    wv1_d = nc.dram_tensor("wv1", [P, L * m4], dt.float32, kind="ExternalInput").ap()
    # output, parity-split: [b, H, parity(0=odd,1=even), T/2]
    out_d = nc.dram_tensor("out", [bloc, H, 2, th], dt.float16,
                           kind="ExternalOutput").ap()

    with tile.TileContext(nc, trace_sim=trace_sim) as tc, ExitStack() as ctx:
        wpool = ctx.enter_context(tc.tile_pool(name="weights", bufs=1))
        xpool = ctx.enter_context(tc.tile_pool(name="xin", bufs=ki * bloc))
        hpool = ctx.enter_context(tc.tile_pool(name="h", bufs=20))
        rpool = ctx.enter_context(tc.tile_pool(name="r", bufs=2))
        tpool = ctx.enter_context(tc.tile_pool(name="t", bufs=3))
        upool = ctx.enter_context(tc.tile_pool(name="u", bufs=2))
        dpool = ctx.enter_context(tc.tile_pool(name="dloc", bufs=3))
        mpool = ctx.enter_context(tc.tile_pool(name="mmin", bufs=3))
        spool = ctx.enter_context(tc.tile_pool(name="stage", bufs=2))
        psum = ctx.enter_context(tc.tile_pool(name="psum", bufs=2, space="PSUM"))

        # ---- persistent weights (small scan/diag operands first, batched
        # into single DMAs, so the DVE/ACT preamble claimers run early) ----
        idm = wpool.tile([P, P], dt.float16, tag="idm")
        nc.gpsimd.dma_start(out=idm[:], in_=idm_d)
        w2t = wpool.tile([P, L * m4], dt.float32, tag="w2t")
        w1t = wpool.tile([P, L * m4], dt.float32, tag="w1t")
        dgt = wpool.tile([P, L * m4 * P], dt.float16, tag="dgt")
        nc.gpsimd.dma_start(out=w2t[:], in_=w2_d)
        nc.gpsimd.dma_start(out=w1t[:], in_=wv1_d)
        nc.gpsimd.dma_start(out=dgt[:], in_=dg_d)
        dg, w2, w1 = [], [], []
        for l in range(L):
            td, tv, tv1 = [], [], []
            for m in range(m4):
                i = l * m4 + m
                td.append(dgt[:, i * P:(i + 1) * P])
                tv.append(w2t[:, i:i + 1])
                tv1.append(w1t[:, i:i + 1])
            dg.append(td)
            w2.append(tv)
            w1.append(tv1)
        wt = []   # wt[l][k] -> [128, H] fp16 lhsT
        for l in range(L):
            kprev = ki if l == 0 else kh
            tw = []
            for k in range(kprev):
                w = wpool.tile([P, H], dt.float16, tag=f"w{l}{k}")
                if l == 0:
                    nc.gpsimd.dma_start(out=w[:], in_=w0t_d[k * P:(k + 1) * P, :])
                else:
                    nc.gpsimd.dma_start(out=w[:], in_=wst_d[l - 1, k * P:(k + 1) * P, :])
                tw.append(w)
            wt.append(tw)
        # layer-0 inputs, all batches up front
        xe0, xo0 = [], []
        first_x_dma = None
        for b in range(bloc):
            te, to = [], []
            for k in range(ki):
                e = xpool.tile([P, th], dt.float16, tag="xe")
                o = xpool.tile([P, th], dt.float16, tag="xo")
                dd = nc.gpsimd.dma_start(out=e[:], in_=xe_d[b, k * P:(k + 1) * P, :])
                if first_x_dma is None:
                    first_x_dma = dd
                nc.gpsimd.dma_start(out=o[:], in_=xo_d[b, k * P:(k + 1) * P, :])
                te.append(e)
                to.append(o)
            xe0.append(te)
            xo0.append(to)

        # Non-PE instructions carry only ONE sync-wait through walrus codegen.
        # Same-engine waits merge, so each engine first "claims" every
        # DMA-loaded operand it will read, leaving real ops a single wait.
        scratch = wpool.tile([P, L * m4], dt.float32, tag="scratch")
        scr_act = wpool.tile([P, 2 * L * m4], dt.float32, tag="scr_act")
        for l in range(L):
            for m in range(m4):
                col = slice(l * m4 + m, l * m4 + m + 1)
                col2 = slice(L * m4 + l * m4 + m, L * m4 + l * m4 + m + 1)
                nc.vector.tensor_copy(scratch[:, col], w2[l][m])
                nc.scalar.activation(scr_act[:, col], w2[l][m], Act.Relu)
                nc.scalar.activation(scr_act[:, col2], w1[l][m], Act.Relu)
        # rotating ACT-claimer scratch (fixed column would WAW itself)
        scr_rot = wpool.tile([P, 6 * bloc * L * m4], dt.float32, tag="scr_rot")
        # PE preamble: junk ldweights per weight tile so later real matmuls
        # never carry a weight-DMA wait.
        for l in range(L):
            for k in range(len(wt[l])):
                nc.tensor.ldweights(weights=wt[l][k][:, 0:P])
            for m in range(m4):
                nc.tensor.ldweights(weights=dg[l][m])
        nc.tensor.ldweights(weights=idm[:])

        # PE warmup: ~4us of junk matmuls so the tensor array reaches its
        # full p-state clock before the first real projection
        wrm = psum.tile([P, TH], dt.float32, tag="py")
        for i in range(18):
            wmm = nc.tensor.matmul(wrm[:, 0:FC], lhsT=wt[0][0][:, 0:P],
                                   rhs=wt[0][0][:, 0:FC], start=True,
                                   stop=True)
            if i == 0:
                bass._add_dep_helper(wmm.ins, first_x_dma.ins, sync=True,
                                     reason="warmup fires as x loads land")
        # ---- main loop: layer-outer, batch-inner for cross-unit pipelining;
        # the last layer runs m-outer so its 2 staging tiles double-buffer.
        # Each unit's tail (block C, h_odd matmuls, combined h relu, stores)
        # is emitted inside the NEXT unit so PE never queues behind the scans.
        h = {}        # (b, l, m) -> combined [h_odd | h_even] tile [P, 2*th]
        ho_ins = {}   # (b, l) -> last DVE h_odd instruction
        he_ins = {}   # (b, l) -> last ACT h_even instruction
        stages = {}
        state = {"cnt": 0, "ci": 0, "last_mm": None, "scan1": None,
                 "last_dve": None, "last_act": None, "last_pe": None,
                 "tail": None}
        px_readers = {}  # px slot / 4: r slot -> last reader instruction

        def dve(ins):
            # pin DVE queue order (sync=False deps cost no sem waits)
            if state["last_dve"] is not None:
                bass._add_dep_helper(ins.ins, state["last_dve"].ins,
                                     sync=False, reason="DVE program order")
            state["last_dve"] = ins
            return ins

        def act(ins):
            if state["last_act"] is not None:
                bass._add_dep_helper(ins.ins, state["last_act"].ins,
                                     sync=False, reason="ACT program order")
            state["last_act"] = ins
            return ins

        def pe(ins):
            if state["last_pe"] is not None:
                bass._add_dep_helper(ins.ins, state["last_pe"].ins,
                                     sync=False, reason="PE program order")
            state["last_pe"] = ins
            return ins

        def act_claim(dep, vec):
            ci = state["ci"]
            c = act(nc.scalar.activation(scr_rot[:, ci:ci + 1],
                                         vec, Act.Relu))
            state["ci"] = ci + 1
            if dep is not None:
                bass._add_dep_helper(c.ins, dep.ins, sync=True,
                                     reason="ACT claimer")
            return c

        def unit(l, b, m):
            kprev = ki if l == 0 else kh
            if l == 0:
                rhs_e = [xt[:] for xt in xe0[b]]
                rhs_o = [xt[:] for xt in xo0[b]]
            else:
                rhs_e = [h[(b, l - 1, k)][:, th:2 * th] for k in range(kh)]
                rhs_o = [h[(b, l - 1, k)][:, 0:th] for k in range(kh)]
            ms = slice(m * P, (m + 1) * P)
            # two psum tiles; py frees right after scan1 so the next unit's
            # odd-projection can start while this unit is still scanning
            py = psum.tile([P, th], dt.float32, tag="py")
            pe_ = psum.tile([P, th], dt.float32, tag="pe")
            slot = state["cnt"] % 2
            state["cnt"] += 1
            claimers = []
            for sl, rd in ((slot, px_readers.get(slot)),
                           (2 + slot, px_readers.get(2 + slot))):
                if rd is not None:
                    ldw = pe(nc.tensor.ldweights(weights=wt[l][0][:, 0:P]))
                    bass._add_dep_helper(ldw.ins, rd.ins, sync=True,
                                         reason="PE claimer for PSUM slot WAR")
                    claimers.append(ldw)
            if m == 0:
                # absorb rhs producer ticks (DMA for l0; DVE h_odd and ACT
                # h_even of (b, l-1) otherwise)
                if l == 0:
                    for xt in (*xe0[b], *xo0[b]):
                        claimers.append(pe(nc.tensor.ldweights(
                            weights=xt[:, 0:P])))
                else:
                    for dep in (ho_ins[(b, l - 1)], he_ins[(b, l - 1)]):
                        ldw = pe(nc.tensor.ldweights(weights=wt[l][0][:, 0:P]))
                        bass._add_dep_helper(
                            ldw.ins, dep.ins, sync=True,
                            reason="PE claimer for rhs producers")
                        claimers.append(ldw)
            # PE block A: py = W.x_o (xp_odd), pe = W.x_e
            first = True
            last_g1 = None
            for f in range(nf):
                fs = slice(f * FC, (f + 1) * FC)
                for k in range(kprev):
                    mm = pe(nc.tensor.matmul(
                        py[:, fs], lhsT=wt[l][k][:, ms], rhs=rhs_o[k][:, fs],
                        start=(k == 0), stop=(k == kprev - 1)))
                    if first:
                        for cl in claimers:
                            bass._add_dep_helper(
                                mm.ins, cl.ins, sync=False,
                                reason="order claimers before MMs")
                        first = False
                    last_g1 = mm
            for f in range(nf):
                fs = slice(f * FC, (f + 1) * FC)
                for k in range(kprev):
                    pe(nc.tensor.matmul(
                        pe_[:, fs], lhsT=wt[l][k][:, ms], rhs=rhs_e[k][:, fs],
                        start=(k == 0), stop=(k == kprev - 1)))
            # ACT r = relu(xp_odd), t = w (*) xp_e
            r = rpool.tile([P, th], dt.float16, tag="r")
            act_claim(px_readers.get(4 + slot), w2[l][m])
            if l == L - 1:
                # no h_even ACT op window on the last layer; absorb the PE
                # group-1 tick so r keeps a single wait
                act_claim(last_g1, w1[l][m])
            r_ins = act(nc.scalar.activation(r[:], py[:], Act.Relu))
            t_ = tpool.tile([P, th], dt.float16, tag="t")
            t_ins = act(nc.scalar.activation(t_[:], pe_[:], Act.Copy,
                                             scale=w1[l][m]))
            # previous unit's tail PE block: overlaps this unit's ACT stage
            if state["tail"] is not None:
                state["tail"][0]()
            # PE block B: py += I.t  -> y
            ldw = pe(nc.tensor.ldweights(weights=idm[:]))
            bass._add_dep_helper(ldw.ins, t_ins.ins, sync=True,
                                 reason="PE claimer: y-accum waits ACT reads")
            first = True
            for f in range(nf):
                fs = slice(f * FC, (f + 1) * FC)
                mm = pe(nc.tensor.matmul(
                    py[:, fs], lhsT=idm[:], rhs=t_[:, fs],
                    start=False, stop=True, skip_group_check=True))
                if first:
                    bass._add_dep_helper(mm.ins, ldw.ins, sync=False,
                                         reason="order claimer before MMs")
                    first = False
                state["last_mm"] = mm
            # previous unit's tail rest (h writes / stores)
            if state["tail"] is not None:
                state["tail"][1]()
                state["tail"] = None
            # DVE scan 1: dloc; out at col 2 so fp16 reads stay 4B-aligned
            dloc = dpool.tile([P, th + 2], dt.float16, tag="dloc")
            ms0 = dve(nc.vector.memset(dloc[:, 1:2], 0.0))
            bass._add_dep_helper(ms0.ins, r_ins.ins, sync=True,
                                 reason="DVE claimer: ACT r tick")
            wv = w2[l][m].broadcast_to((P, th))
            scan1 = dve(nc.vector.tensor_tensor_scan(
                out=dloc[:, 2:th + 2], data0=wv, data1=py[:],
                initial=0.0, op0=Alu.mult, op1=Alu.add))
            state["scan1"] = scan1
            px_readers[slot] = scan1
            # DVE u' = r - dloc (feeds the max-scan for M' = -M)
            u = upool.tile([P, th], dt.float16, tag="u")
            u_ins = dve(nc.vector.tensor_tensor(
                out=u[:], in0=r[:], in1=dloc[:, 2:th + 2], op=Alu.subtract))
            px_readers[4 + slot] = u_ins
            # DVE scan 2: M'[k+1] = max(w^2 M'[k], u'[k])
            mt = mpool.tile([P, th + 2], dt.float16, tag="mmin")
            dve(nc.vector.memset(mt[:, 1:2], 0.0))
            dve(nc.vector.tensor_tensor_scan(
                out=mt[:, 2:th + 2], data0=wv, data1=u[:],
                initial=0.0, op0=Alu.mult, op1=Alu.max))
            cbox = {}

            def tail_pe():
                # block C: pe += diag(w).dloc_shift + diag(w).M'_shift
                for f in range(nf):
                    fs = slice(f * FC, (f + 1) * FC)
                    pe(nc.tensor.matmul(
                        pe_[:, fs], lhsT=dg[l][m],
                        rhs=dloc[:, 1 + f * FC:1 + f * FC + FC],
                        start=False, stop=False, skip_group_check=True))
                    cbox["c"] = pe(nc.tensor.matmul(
                        pe_[:, fs], lhsT=dg[l][m],
                        rhs=mt[:, 1 + f * FC:1 + f * FC + FC],
                        start=False, stop=True, skip_group_check=True))
                    state["last_mm"] = cbox["c"]

            def tail_rest():
                if l < L - 1:
                    ht = hpool.tile([P, 2 * th], dt.float16, tag="h")
                    # h_odd = dloc + M' on DVE into ht[0:th]
                    ho_i = dve(nc.vector.tensor_tensor(
                        out=ht[:, 0:th], in0=dloc[:, 2:th + 2],
                        in1=mt[:, 2:th + 2], op=Alu.add))
                    # ACT claimer: absorb the PE block-C tick so h_even
                    # keeps only its own-engine ordering wait
                    act_claim(cbox["c"], w2[l][m])
                    he_i = act(nc.scalar.activation(ht[:, th:2 * th], pe_[:],
                                                    Act.Relu))
                    px_readers[2 + slot] = he_i
                    ho_ins[(b, l)] = ho_i
                    he_ins[(b, l)] = he_i
                    h[(b, l, m)] = ht
                else:
                    # final layer: both halves written by DVE into the
                    # batch-pair staging tile; h_even read directly from
                    # PSUM so the store carries a single DVE wait
                    if b % 2 == 0:
                        st = spool.tile([P, 2 * t], dt.float16, tag="stage")
                        stages[m] = st
                        # first toucher claims the store-DMA WAR tick
                        dve(nc.vector.memset(st[:, 0:1], 0.0))
                    st = stages[m]
                    off = (b % 2) * t
                    cs = state["ci"]
                    cdve = dve(nc.vector.memset(
                        scratch[:, (cs % (L * m4)):(cs % (L * m4)) + 1], 0.0))
                    bass._add_dep_helper(cdve.ins, cbox["c"].ins, sync=True,
                                         reason="DVE claimer: PE blockC tick")
                    ev = dve(nc.vector.tensor_scalar_max(
                        st[:, off + th:off + t], pe_[:], 0.0))
                    px_readers[2 + slot] = ev
                    odd = dve(nc.vector.tensor_tensor(
                        out=st[:, off:off + th], in0=dloc[:, 2:th + 2],
                        in1=mt[:, 2:th + 2], op=Alu.add))
                    if b % 2 == 1:
                        dst = out_d[b - 1:b + 1, ms, :, :]
                        nc.sync.dma_start(
                            out=dst.rearrange("b p r t -> p b r t"),
                            in_=st[:].rearrange("p (b r t) -> p b r t",
                                                b=2, r=2))
            state["tail"] = (tail_pe, tail_rest)

        for l in range(L - 1):
            for b in range(bloc):
                for m in range(m4):
                    unit(l, b, m)
        for m in range(m4):
            for b in range(bloc):
                unit(L - 1, b, m)
        state["tail"][0]()
        # final flush has no successor unit to absorb the last scan tick
        act_claim(state["scan1"], w2[0][0])
        state["tail"][1]()
        state["tail"] = None

        # ---- tail pre-drain (see baseline): absorb every DMA queue and
        # engine tick so the auto kernel-tail drain ends at zero waits.
        tail_deps = [i for i in nc.inst_map.values()
                     if type(i).__name__ == "InstDMACopy"]
        snap = list(nc.inst_map.values())
        compute_tys = {"InstTensorScalarPtr", "InstTensorTensor",
                       "InstActivation", "InstTensorCopy", "InstMemset"}
        for eng in ("DVE", "Activation"):
            last_e = [i for i in snap
                      if str(getattr(i, "engine", "")).endswith(eng)
                      and type(i).__name__ in compute_tys]
            if last_e:
                tail_deps.append(last_e[-1])
        tail_deps += [state["last_mm"].ins, state["scan1"].ins]
        for depi in tail_deps:
            dr = nc.sync.drain(fusable=False)
            bass._add_dep_helper(dr.ins, depi, sync=True,
                                 reason="tail pre-drain absorber")
    _assert_wait_budget(nc)
    return nc


_MULTI_WAIT_OK = {"InstDrain",
                  "InstEventSemaphore", "InstUnconditionalBranch",
                  "InstRegisterMove", "InstISA", "InstTensorLoad",
                  "InstTensorSave"}


def _assert_wait_budget(nc):
    bad = []
    for name, inst in nc.inst_map.items():
        ty = type(inst).__name__
        if ty in _MULTI_WAIT_OK:
            continue
        w = inst.sync_info.on_wait if inst.sync_info else []
        if len(w) > 1:
            bad.append((name, ty,
                        [f"{x.ant_name}>={x.wait_value}" for x in w]))
    if bad:
        raise RuntimeError(
            f"{len(bad)} instructions exceed the 1-sync-wait TPB limit, "
            f"first few: {bad[:5]}")


def _prep_core_inputs(Input, W0, Ws, bs, whs, core):
    """Host-side staging for one core: shard batch, transpose + parity-split
    the layer-0 input, lhsT weights, diag matrices, w and w^2 vectors."""
    bsl = slice(core * BLOC, (core + 1) * BLOC)
    xT = Input[bsl].transpose(0, 2, 1).astype(np.float16)  # [bloc, I, T]
    w0t = W0.T.astype(np.float16)                          # [I, H]
    wst = Ws.transpose(0, 2, 1).astype(np.float16)         # [L-1, H, H]
    whsf = whs.astype(np.float32)                          # [L, H]
    m4 = H // P
    dgm = np.zeros((L, m4, P, P), np.float16)
    for l in range(L):
        for m in range(m4):
            blk = whsf[l, m * P:(m + 1) * P]
            np.fill_diagonal(dgm[l, m], blk.astype(np.float16))
    # partition-major small operands: [p, l*m4(*P)] so each loads in one DMA
    dg_pm = np.ascontiguousarray(
        dgm.transpose(2, 0, 1, 3).reshape(P, L * m4 * P))
    w2_pm = np.ascontiguousarray(
        (whsf * whsf).reshape(L, m4, P).transpose(2, 0, 1).reshape(P, L * m4))
    w1_pm = np.ascontiguousarray(
        whsf.reshape(L, m4, P).transpose(2, 0, 1).reshape(P, L * m4))
    return {
        "xe": np.ascontiguousarray(xT[:, :, 0::2]),
        "xo": np.ascontiguousarray(xT[:, :, 1::2]),
        "w0t": np.ascontiguousarray(w0t),
        "wst": np.ascontiguousarray(wst),
        "idm": np.eye(P, dtype=np.float16),
        "dg": dg_pm,
        "w2": w2_pm,
        "wv1": w1_pm,
    }


def kernel(Input, W0, Ws, bs, whs):
    include_bias = bool(np.any(bs != 0))
    nc = build(include_bias=include_bias)
    in_maps = [_prep_core_inputs(Input, W0, Ws, bs, whs, r)
               for r in range(NCORES)]
    res = run_bass_kernel_spmd(nc, in_maps, core_ids=list(range(NCORES)))
    parts = [res.results[r]["out"] for r in range(NCORES)]  # [BLOC, H, 2, T/2]
    po = np.concatenate(parts, axis=0)  # [B, H, 2, T/2]; 0=odd, 1=even
    full = np.empty((B, H, T), np.float16)
    full[:, :, 1::2] = po[:, :, 0, :]
    full[:, :, 0::2] = po[:, :, 1, :]
    return np.ascontiguousarray(full.transpose(0, 2, 1)).astype(np.float32)


# revision 37
# speedup vs baseline: 1.8246x; 1.0083x over previous
"""Trainium2 Bass kernel for a 4-layer IndRNN (B=32, T=2048, I=256, H=512).

Math per layer: xp = x @ W.T (+b), then h_t = relu(xp_t + w (*) h_{t-1}),
with per-channel recurrent weight w = whs[l] in [0, 1).

The nonlinear scan decomposes into two linear-style DVE scans (see the
baseline derivation): dloc = linear scan of xp with factor w, q = min-scan,
h = relu(dloc - q). DVE scans cost ~2.1 ns/element regardless of dtype, so
this kernel additionally DECIMATES TIME BY 2: both scans run at length T/2
over pair-combined inputs, and the other parity is recovered on PE/ACT.

Per channel (K = T/2), validated exactly in fp64 (sim_check.py):
    xp_e[k] = xp[2k], xp_o[k] = xp[2k+1]
    y[k]        = w*xp_e[k] + xp_o[k]        (PE proj + ACT scale + PE I-accum)
    dloc[k]     = w^2*dloc[k-1] + y[k]       (DVE scan 1, length T/2)
    u'[k]       = relu(xp_o[k]) - dloc[k]    (ACT relu + DVE subtract)
    M'[k+1]     = max(w^2*M'[k], u'[k])      (DVE scan 2;  M' = -M)
    h_odd[k]    = dloc[k] + M'[k+1]          (PE identity-matmul accumulate)
    dloc_e[k]   = w*dloc[k-1] + xp_e[k]      (PE diag-matmul accumulate)
    h_even[k]   = relu(dloc_e[k] + w*M'[k])
Since h_odd >= 0 (it is a post-relu state), relu is a no-op on it, so ONE
ACT relu over the combined PSUM region [h_odd_pre | h_even_pre] materializes
both parities in a single pass.

Only the two scans and the u' subtract run on the DVE (~5.4us per
128-channel tile); everything else lives on PE/ACT, software-pipelined one
unit ahead so PE never queues behind the scans.

Sharding: data-parallel over batch, 4 batches per core, weights replicated.
Layout on device: [H(partitions), T/2(free)] per parity per batch; the host
pre-splits time parities and pre-transposes, and re-interleaves on the way
out, so the device never pays for transposes or strided DMA.
"""

import numpy as np
from contextlib import ExitStack

import concourse.bass as bass
import concourse.tile as tile
from concourse import mybir
from concourse.bass_utils import run_bass_kernel_spmd

dt = mybir.dt
Alu = mybir.AluOpType
Act = mybir.ActivationFunctionType

B, T, I, H, L = 32, 2048, 256, 512, 4
NCORES = 8
BLOC = B // NCORES
P = 128
TH = T // 2          # decimated scan length
FC = 512             # matmul free-dim chunk (= one PSUM bank of fp32)


def build(bloc=BLOC, t=T, include_bias=False, trace_sim=False):
    """Build the per-core Bass program (SPMD; identical on all cores)."""
    assert not include_bias, "bias path not implemented (bs==0 in this problem)"
    th = t // 2
    nf = th // FC
    ki, kh, m4 = I // P, H // P, H // P

    nc = bass.Bass("TRN2", target_bir_lowering=False, debug=False,
                   num_devices=NCORES)
    xe_d = nc.dram_tensor("xe", [bloc, I, th], dt.float16, kind="ExternalInput").ap()
    xo_d = nc.dram_tensor("xo", [bloc, I, th], dt.float16, kind="ExternalInput").ap()
    w0t_d = nc.dram_tensor("w0t", [I, H], dt.float16, kind="ExternalInput").ap()
    wst_d = nc.dram_tensor("wst", [L - 1, H, H], dt.float16, kind="ExternalInput").ap()
    idm_d = nc.dram_tensor("idm", [P, P], dt.float16, kind="ExternalInput").ap()
    dg_d = nc.dram_tensor("dg", [P, L * m4 * P], dt.float16, kind="ExternalInput").ap()
    w2_d = nc.dram_tensor("w2", [P, L * m4], dt.float32, kind="ExternalInput").ap()
    wv1_d = nc.dram_tensor("wv1", [P, L * m4], dt.float32, kind="ExternalInput").ap()
    # output, parity-split: [b, H, parity(0=odd,1=even), T/2]
    out_d = nc.dram_tensor("out", [bloc, H, 2, th], dt.float16,
                           kind="ExternalOutput").ap()

    with tile.TileContext(nc, trace_sim=trace_sim) as tc, ExitStack() as ctx:
        wpool = ctx.enter_context(tc.tile_pool(name="weights", bufs=1))
        xpool = ctx.enter_context(tc.tile_pool(name="xin", bufs=ki * bloc))
        hpool = ctx.enter_context(tc.tile_pool(name="h", bufs=20))
        rpool = ctx.enter_context(tc.tile_pool(name="r", bufs=2))
        tpool = ctx.enter_context(tc.tile_pool(name="t", bufs=3))
        upool = ctx.enter_context(tc.tile_pool(name="u", bufs=2))
        dpool = ctx.enter_context(tc.tile_pool(name="dloc", bufs=3))
        mpool = ctx.enter_context(tc.tile_pool(name="mmin", bufs=3))
        spool = ctx.enter_context(tc.tile_pool(name="stage", bufs=2))
        psum = ctx.enter_context(tc.tile_pool(name="psum", bufs=2, space="PSUM"))

        # ---- persistent weights (small scan/diag operands first, batched
        # into single DMAs, so the DVE/ACT preamble claimers run early) ----
        idm = wpool.tile([P, P], dt.float16, tag="idm")
        nc.gpsimd.dma_start(out=idm[:], in_=idm_d)
        w2t = wpool.tile([P, L * m4], dt.float32, tag="w2t")
        w1t = wpool.tile([P, L * m4], dt.float32, tag="w1t")
        dgt = wpool.tile([P, L * m4 * P], dt.float16, tag="dgt")
        nc.gpsimd.dma_start(out=w2t[:], in_=w2_d)
        nc.gpsimd.dma_start(out=w1t[:], in_=wv1_d)
        nc.gpsimd.dma_start(out=dgt[:], in_=dg_d)
        dg, w2, w1 = [], [], []
        for l in range(L):
            td, tv, tv1 = [], [], []
            for m in range(m4):
                i = l * m4 + m
                td.append(dgt[:, i * P:(i + 1) * P])
                tv.append(w2t[:, i:i + 1])
                tv1.append(w1t[:, i:i + 1])
            dg.append(td)
            w2.append(tv)
            w1.append(tv1)
        wt = []   # wt[l][k] -> [128, H] fp16 lhsT
        for l in range(L):
            kprev = ki if l == 0 else kh
            tw = []
            for k in range(kprev):
                w = wpool.tile([P, H], dt.float16, tag=f"w{l}{k}")
                if l == 0:
                    nc.gpsimd.dma_start(out=w[:], in_=w0t_d[k * P:(k + 1) * P, :])
                else:
                    nc.gpsimd.dma_start(out=w[:], in_=wst_d[l - 1, k * P:(k + 1) * P, :])
                tw.append(w)
            wt.append(tw)
        # layer-0 inputs, all batches up front
        xe0, xo0 = [], []
        first_x_dma = None
        for b in range(bloc):
            te, to = [], []
            for k in range(ki):
                e = xpool.tile([P, th], dt.float16, tag="xe")
                o = xpool.tile([P, th], dt.float16, tag="xo")
                dd = nc.gpsimd.dma_start(out=e[:], in_=xe_d[b, k * P:(k + 1) * P, :])
                if first_x_dma is None:
                    first_x_dma = dd
                nc.gpsimd.dma_start(out=o[:], in_=xo_d[b, k * P:(k + 1) * P, :])
                te.append(e)
                to.append(o)
            xe0.append(te)
            xo0.append(to)

        # Non-PE instructions carry only ONE sync-wait through walrus codegen.
        # Same-engine waits merge, so each engine first "claims" every
        # DMA-loaded operand it will read, leaving real ops a single wait.
        scratch = wpool.tile([P, L * m4], dt.float32, tag="scratch")
        scr_act = wpool.tile([P, 2 * L * m4], dt.float32, tag="scr_act")
        for l in range(L):
            for m in range(m4):
                col = slice(l * m4 + m, l * m4 + m + 1)
                col2 = slice(L * m4 + l * m4 + m, L * m4 + l * m4 + m + 1)
                nc.vector.tensor_copy(scratch[:, col], w2[l][m])
                nc.scalar.activation(scr_act[:, col], w2[l][m], Act.Relu)
                nc.scalar.activation(scr_act[:, col2], w1[l][m], Act.Relu)
        # rotating ACT-claimer scratch (fixed column would WAW itself)
        scr_rot = wpool.tile([P, 6 * bloc * L * m4], dt.float32, tag="scr_rot")
        # PE preamble: junk ldweights per weight tile so later real matmuls
        # never carry a weight-DMA wait.
        for l in range(L):
            for k in range(len(wt[l])):
                nc.tensor.ldweights(weights=wt[l][k][:, 0:P])
            for m in range(m4):
                nc.tensor.ldweights(weights=dg[l][m])
        nc.tensor.ldweights(weights=idm[:])

        # PE warmup: ~4us of junk matmuls so the tensor array reaches its
        # full p-state clock before the first real projection
        wrm = psum.tile([P, TH], dt.float32, tag="py")
        for i in range(18):
            wmm = nc.tensor.matmul(wrm[:, 0:FC], lhsT=wt[0][0][:, 0:P],
                                   rhs=wt[0][0][:, 0:FC], start=True,
                                   stop=True)
            if i == 0:
                bass._add_dep_helper(wmm.ins, first_x_dma.ins, sync=True,
                                     reason="warmup fires as x loads land")
        # ---- main loop: layer-outer, batch-inner for cross-unit pipelining;
        # the last layer runs m-outer so its 2 staging tiles double-buffer.
        # Each unit's tail (block C, h_odd matmuls, combined h relu, stores)
        # is emitted inside the NEXT unit so PE never queues behind the scans.
        h = {}        # (b, l, m) -> combined [h_odd | h_even] tile [P, 2*th]
        ho_ins = {}   # (b, l) -> last DVE h_odd instruction
        he_ins = {}   # (b, l) -> last ACT h_even instruction
        stages = {}
        state = {"cnt": 0, "ci": 0, "last_mm": None, "scan1": None,
                 "last_dve": None, "last_act": None, "last_pe": None,
                 "tail": None}
        px_readers = {}  # px slot / 4: r slot -> last reader instruction

        def dve(ins):
            # pin DVE queue order (sync=False deps cost no sem waits)
            if state["last_dve"] is not None:
                bass._add_dep_helper(ins.ins, state["last_dve"].ins,
                                     sync=False, reason="DVE program order")
            state["last_dve"] = ins
            return ins

        def act(ins):
            if state["last_act"] is not None:
                bass._add_dep_helper(ins.ins, state["last_act"].ins,
                                     sync=False, reason="ACT program order")
            state["last_act"] = ins
            return ins

        def pe(ins):
            if state["last_pe"] is not None:
                bass._add_dep_helper(ins.ins, state["last_pe"].ins,
                                     sync=False, reason="PE program order")
            state["last_pe"] = ins
            return ins

        def act_claim(dep, vec):
            ci = state["ci"]
            c = act(nc.scalar.activation(scr_rot[:, ci:ci + 1],
                                         vec, Act.Relu))
            state["ci"] = ci + 1
            if dep is not None:
                bass._add_dep_helper(c.ins, dep.ins, sync=True,
                                     reason="ACT claimer")
            return c

        def unit(l, b, m):
            kprev = ki if l == 0 else kh
            if l == 0:
                rhs_e = [xt[:] for xt in xe0[b]]
                rhs_o = [xt[:] for xt in xo0[b]]
            else:
                rhs_e = [h[(b, l - 1, k)][:, th:2 * th] for k in range(kh)]
                rhs_o = [h[(b, l - 1, k)][:, 0:th] for k in range(kh)]
            ms = slice(m * P, (m + 1) * P)
            # two psum tiles; py frees right after scan1 so the next unit's
            # odd-projection can start while this unit is still scanning
            py = psum.tile([P, th], dt.float32, tag="py")
            pe_ = psum.tile([P, th], dt.float32, tag="pe")
            slot = state["cnt"] % 2
            state["cnt"] += 1
            claimers = []
            for sl, rd in ((slot, px_readers.get(slot)),
                           (2 + slot, px_readers.get(2 + slot))):
                if rd is not None:
                    ldw = pe(nc.tensor.ldweights(weights=wt[l][0][:, 0:P]))
                    bass._add_dep_helper(ldw.ins, rd.ins, sync=True,
                                         reason="PE claimer for PSUM slot WAR")
                    claimers.append(ldw)
            if m == 0:
                # absorb rhs producer ticks (DMA for l0; DVE h_odd and ACT
                # h_even of (b, l-1) otherwise)
                if l == 0:
                    for xt in (*xe0[b], *xo0[b]):
                        claimers.append(pe(nc.tensor.ldweights(
                            weights=xt[:, 0:P])))
                else:
                    for dep in (ho_ins[(b, l - 1)], he_ins[(b, l - 1)]):
                        ldw = pe(nc.tensor.ldweights(weights=wt[l][0][:, 0:P]))
                        bass._add_dep_helper(
                            ldw.ins, dep.ins, sync=True,
                            reason="PE claimer for rhs producers")
                        claimers.append(ldw)
            # PE block A: py = W.x_o (xp_odd), pe = W.x_e
            first = True
            last_g1 = None
            for f in range(nf):
                fs = slice(f * FC, (f + 1) * FC)
                for k in range(kprev):
                    mm = pe(nc.tensor.matmul(
                        py[:, fs], lhsT=wt[l][k][:, ms], rhs=rhs_o[k][:, fs],
                        start=(k == 0), stop=(k == kprev - 1)))
                    if first:
                        for cl in claimers:
                            bass._add_dep_helper(
                                mm.ins, cl.ins, sync=False,
                                reason="order claimers before MMs")
                        first = False
                    last_g1 = mm
            for f in range(nf):
                fs = slice(f * FC, (f + 1) * FC)
                for k in range(kprev):
                    pe(nc.tensor.matmul(
                        pe_[:, fs], lhsT=wt[l][k][:, ms], rhs=rhs_e[k][:, fs],
                        start=(k == 0), stop=(k == kprev - 1)))
            # ACT r = relu(xp_odd), t = w (*) xp_e
            r = rpool.tile([P, th], dt.float16, tag="r")
            act_claim(px_readers.get(4 + slot), w2[l][m])
            if l == L - 1:
                # no h_even ACT op window on the last layer; absorb the PE
                # group-1 tick so r keeps a single wait
                act_claim(last_g1, w1[l][m])
            r_ins = act(nc.scalar.activation(r[:], py[:], Act.Relu))
            t_ = tpool.tile([P, th], dt.float16, tag="t")
            t_ins = act(nc.scalar.activation(t_[:], pe_[:], Act.Copy,
                                             scale=w1[l][m]))
            # previous unit's tail PE block: overlaps this unit's ACT stage
            if state["tail"] is not None:
                state["tail"][0]()
            # PE block B: py += I.t  -> y
            ldw = pe(nc.tensor.ldweights(weights=idm[:]))
            bass._add_dep_helper(ldw.ins, t_ins.ins, sync=True,
                                 reason="PE claimer: y-accum waits ACT reads")
            first = True
            for f in range(nf):
                fs = slice(f * FC, (f + 1) * FC)
                mm = pe(nc.tensor.matmul(
                    py[:, fs], lhsT=idm[:], rhs=t_[:, fs],
                    start=False, stop=True, skip_group_check=True))
                if first:
                    bass._add_dep_helper(mm.ins, ldw.ins, sync=False,
                                         reason="order claimer before MMs")
                    first = False
                state["last_mm"] = mm
            # previous unit's tail rest (h writes / stores)
            if state["tail"] is not None:
                state["tail"][1]()
                state["tail"] = None
            # DVE scan 1: dloc; out at col 2 so fp16 reads stay 4B-aligned
            dloc = dpool.tile([P, th + 2], dt.float16, tag="dloc")
            ms0 = dve(nc.vector.memset(dloc[:, 1:2], 0.0))
            bass._add_dep_helper(ms0.ins, r_ins.ins, sync=True,
                                 reason="DVE claimer: ACT r tick")
            wv = w2[l][m].broadcast_to((P, th))
            scan1 = dve(nc.vector.tensor_tensor_scan(
                out=dloc[:, 2:th + 2], data0=wv, data1=py[:],
                initial=0.0, op0=Alu.mult, op1=Alu.add))
            state["scan1"] = scan1
            px_readers[slot] = scan1
            # DVE u' = r - dloc (feeds the max-scan for M' = -M)
            u = upool.tile([P, th], dt.float16, tag="u")
            u_ins = dve(nc.vector.tensor_tensor(
                out=u[:], in0=r[:], in1=dloc[:, 2:th + 2], op=Alu.subtract))
            px_readers[4 + slot] = u_ins
            # DVE scan 2: M'[k+1] = max(w^2 M'[k], u'[k])
            mt = mpool.tile([P, th + 2], dt.float16, tag="mmin")
            dve(nc.vector.memset(mt[:, 1:2], 0.0))
            dve(nc.vector.tensor_tensor_scan(
                out=mt[:, 2:th + 2], data0=wv, data1=u[:],
                initial=0.0, op0=Alu.mult, op1=Alu.max))
            cbox = {}

            def tail_pe():
                # block C: pe += diag(w).dloc_shift + diag(w).M'_shift
                for f in range(nf):
                    fs = slice(f * FC, (f + 1) * FC)
                    pe(nc.tensor.matmul(
                        pe_[:, fs], lhsT=dg[l][m],
                        rhs=dloc[:, 1 + f * FC:1 + f * FC + FC],
                        start=False, stop=False, skip_group_check=True))
                    cbox["c"] = pe(nc.tensor.matmul(
                        pe_[:, fs], lhsT=dg[l][m],
                        rhs=mt[:, 1 + f * FC:1 + f * FC + FC],
                        start=False, stop=True, skip_group_check=True))
                    state["last_mm"] = cbox["c"]

            def tail_rest():
                if l < L - 1:
                    ht = hpool.tile([P, 2 * th], dt.float16, tag="h")
                    # h_odd = dloc + M' on DVE into ht[0:th]
                    ho_i = dve(nc.vector.tensor_tensor(
                        out=ht[:, 0:th], in0=dloc[:, 2:th + 2],
                        in1=mt[:, 2:th + 2], op=Alu.add))
                    # ACT claimer: absorb the PE block-C tick so h_even
                    # keeps only its own-engine ordering wait
                    act_claim(cbox["c"], w2[l][m])
                    he_i = act(nc.scalar.activation(ht[:, th:2 * th], pe_[:],
                                                    Act.Relu))
                    px_readers[2 + slot] = he_i
                    ho_ins[(b, l)] = ho_i
                    he_ins[(b, l)] = he_i
                    h[(b, l, m)] = ht
                else:
                    # final layer: ACT computes h_even into an h tile; DVE
                    # copies it into the staging tile at 4x fp16 rate and
                    # adds the odd half, so the store keeps one DVE wait
                    ht = hpool.tile([P, 2 * th], dt.float16, tag="h")
                    act_claim(cbox["c"], w2[l][m])
                    he_i = act(nc.scalar.activation(ht[:, th:2 * th], pe_[:],
                                                    Act.Relu))
                    px_readers[2 + slot] = he_i
                    if b % 2 == 0:
                        st = spool.tile([P, 2 * t], dt.float16, tag="stage")
                        stages[m] = st
                        # first toucher claims the store-DMA WAR tick
                        dve(nc.vector.memset(st[:, 0:1], 0.0))
                    st = stages[m]
                    off = (b % 2) * t
                    odd = dve(nc.vector.tensor_tensor(
                        out=st[:, off:off + th], in0=dloc[:, 2:th + 2],
                        in1=mt[:, 2:th + 2], op=Alu.add))
                    cp = dve(nc.vector.tensor_copy(st[:, off + th:off + t],
                                                   ht[:, th:2 * th]))
                    bass._add_dep_helper(cp.ins, he_i.ins, sync=True,
                                         reason="DVE copy waits ACT h_even")
                    if b % 2 == 1:
                        dst = out_d[b - 1:b + 1, ms, :, :]
                        nc.sync.dma_start(
                            out=dst.rearrange("b p r t -> p b r t"),
                            in_=st[:].rearrange("p (b r t) -> p b r t",
                                                b=2, r=2))
            state["tail"] = (tail_pe, tail_rest)

        for l in range(L - 1):
            for b in range(bloc):
                for m in range(m4):
                    unit(l, b, m)
        for m in range(m4):
            for b in range(bloc):
                unit(L - 1, b, m)
        state["tail"][0]()
        # final flush has no successor unit to absorb the last scan tick
        act_claim(state["scan1"], w2[0][0])
        state["tail"][1]()
        state["tail"] = None

        # ---- tail pre-drain (see baseline): absorb every DMA queue and
        # engine tick so the auto kernel-tail drain ends at zero waits.
        tail_deps = [i for i in nc.inst_map.values()
                     if type(i).__name__ == "InstDMACopy"]
        snap = list(nc.inst_map.values())
        compute_tys = {"InstTensorScalarPtr", "InstTensorTensor",
                       "InstActivation", "InstTensorCopy", "InstMemset"}
        for eng in ("DVE", "Activation"):
            last_e = [i for i in snap
                      if str(getattr(i, "engine", "")).endswith(eng)
                      and type(i).__name__ in compute_tys]
            if last_e:
                tail_deps.append(last_e[-1])
        tail_deps += [state["last_mm"].ins, state["scan1"].ins]
        for depi in tail_deps:
            dr = nc.sync.drain(fusable=False)
            bass._add_dep_helper(dr.ins, depi, sync=True,
                                 reason="tail pre-drain absorber")
    _assert_wait_budget(nc)
    return nc


_MULTI_WAIT_OK = {"InstDrain",
                  "InstEventSemaphore", "InstUnconditionalBranch",
                  "InstRegisterMove", "InstISA", "InstTensorLoad",
                  "InstTensorSave"}


def _assert_wait_budget(nc):
    bad = []
    for name, inst in nc.inst_map.items():
        ty = type(inst).__name__
        if ty in _MULTI_WAIT_OK:
            continue
        w = inst.sync_info.on_wait if inst.sync_info else []
        if len(w) > 1:
            bad.append((name, ty,
                        [f"{x.ant_name}>={x.wait_value}" for x in w]))
    if bad:
        raise RuntimeError(
            f"{len(bad)} instructions exceed the 1-sync-wait TPB limit, "
            f"first few: {bad[:5]}")


def _prep_core_inputs(Input, W0, Ws, bs, whs, core):
    """Host-side staging for one core: shard batch, transpose + parity-split
    the layer-0 input, lhsT weights, diag matrices, w and w^2 vectors."""
    bsl = slice(core * BLOC, (core + 1) * BLOC)
    xT = Input[bsl].transpose(0, 2, 1).astype(np.float16)  # [bloc, I, T]
    w0t = W0.T.astype(np.float16)                          # [I, H]
    wst = Ws.transpose(0, 2, 1).astype(np.float16)         # [L-1, H, H]
    whsf = whs.astype(np.float32)                          # [L, H]
    m4 = H // P
    dgm = np.zeros((L, m4, P, P), np.float16)
    for l in range(L):
        for m in range(m4):
            blk = whsf[l, m * P:(m + 1) * P]
            np.fill_diagonal(dgm[l, m], blk.astype(np.float16))
    # partition-major small operands: [p, l*m4(*P)] so each loads in one DMA
    dg_pm = np.ascontiguousarray(
        dgm.transpose(2, 0, 1, 3).reshape(P, L * m4 * P))
    w2_pm = np.ascontiguousarray(
        (whsf * whsf).reshape(L, m4, P).transpose(2, 0, 1).reshape(P, L * m4))
    w1_pm = np.ascontiguousarray(
        whsf.reshape(L, m4, P).transpose(2, 0, 1).reshape(P, L * m4))
    return {
        "xe": np.ascontiguousarray(xT[:, :, 0::2]),
        "xo": np.ascontiguousarray(xT[:, :, 1::2]),
        "w0t": np.ascontiguousarray(w0t),
        "wst": np.ascontiguousarray(wst),
        "idm": np.eye(P, dtype=np.float16),
        "dg": dg_pm,
        "w2": w2_pm,
        "wv1": w1_pm,
    }


def kernel(Input, W0, Ws, bs, whs):
    include_bias = bool(np.any(bs != 0))
    nc = build(include_bias=include_bias)
    in_maps = [_prep_core_inputs(Input, W0, Ws, bs, whs, r)
               for r in range(NCORES)]
    res = run_bass_kernel_spmd(nc, in_maps, core_ids=list(range(NCORES)))
    parts = [res.results[r]["out"] for r in range(NCORES)]  # [BLOC, H, 2, T/2]
    po = np.concatenate(parts, axis=0)  # [B, H, 2, T/2]; 0=odd, 1=even
    full = np.empty((B, H, T), np.float16)
    full[:, :, 1::2] = po[:, :, 0, :]
    full[:, :, 0::2] = po[:, :, 1, :]
    return np.ascontiguousarray(full.transpose(0, 2, 1)).astype(np.float32)
